# revision 1
# baseline (speedup 1.0000x reference)
"""EuclRiemGrassAtt fused attention kernel for 8 Trainium2 NeuronCores.

Sharding: core c -> (batch b = c//2, row-half = c%2). Each core computes
512 query rows x 1024 keys for all 8 heads; no inter-core communication.

Device layout trick: scores are computed transposed with a 16-key x 8-head
partition interleave [p = ml*8+h, n] so that the 24->8 BN+conv channel mix,
the softmax denominator and the attention*V contraction are all plain PE
matmuls (contraction over the partition axis).

The Grassmannian QR is reproduced via  Qq @ Qk^T = q @ (Rq^-1 Rk^-T) @ k^T.
The R factors must carry LAPACK's Householder sign convention (the reference
squares Qq@Qk^T elementwise, which is NOT invariant to QR column signs), so
the tiny 32x32 R solves run on host; all O(N^2) work runs on device.
"""

import numpy as np

B, N, C, H, HD = 4, 1024, 256, 8, 32
NH = N // 2          # rows per core
G = N // 16          # 64 key-groups of 16
BN_EPS = 1e-5

_CACHE = {}


def _build_program():
    import concourse.bass as bass
    import concourse.tile as tile
    from concourse import bacc, mybir

    f32 = mybir.dt.float32
    nc = bacc.Bacc(target_bir_lowering=False)

    qt_d = nc.dram_tensor("qt", [2, 128, NH], f32, kind="ExternalInput")
    qpt_d = nc.dram_tensor("qpt", [2, 128, NH], f32, kind="ExternalInput")
    ks_d = nc.dram_tensor("ks", [2, 128, G * 128], f32, kind="ExternalInput")
    vs_d = nc.dram_tensor("vs_in", [128, G * 256], f32, kind="ExternalInput")
    ones8_d = nc.dram_tensor("ones8", [128, 8], f32, kind="ExternalInput")
    w2e_d = nc.dram_tensor("w2e", [128, 128], f32, kind="ExternalInput")
    w2r_d = nc.dram_tensor("w2r", [128, 128], f32, kind="ExternalInput")
    w2g_d = nc.dram_tensor("w2g", [128, 128], f32, kind="ExternalInput")
    biasv_d = nc.dram_tensor("biasv", [128, 1], f32, kind="ExternalInput")
    sel_d = nc.dram_tensor("sel", [2, 8, 128], f32, kind="ExternalInput")
    wpt_d = nc.dram_tensor("wpt", [2, 128, 256], f32, kind="ExternalInput")
    bpj_d = nc.dram_tensor("bpj", [2, 128, 1], f32, kind="ExternalInput")
    yt_d = nc.dram_tensor("yt", [2, 128, NH], f32, kind="ExternalOutput")

    AF = mybir.ActivationFunctionType

    # Stage all constant loads outside TileContext behind ONE explicit
    # semaphore (per-instruction HW sync-wait slots are scarce; a single
    # shared sem costs each engine exactly one wait).
    def sb(name, shape):
        return nc.alloc_sbuf_tensor(name, shape, f32).ap()

    ks0, ks1 = sb("ks0", [128, G * 128]), sb("ks1", [128, G * 128])
    vs = sb("vs", [128, G * 256])
    qt0, qt1 = sb("qt0", [128, NH]), sb("qt1", [128, NH])
    qpt0, qpt1 = sb("qpt0", [128, NH]), sb("qpt1", [128, NH])
    ones8 = sb("ones8s", [128, 8])
    w2e, w2r, w2g = sb("w2es", [128, 128]), sb("w2rs", [128, 128]), sb("w2gs", [128, 128])
    biasv = sb("biasvs", [128, 1])
    sel1, sel2 = sb("sel1s", [8, 128]), sb("sel2s", [8, 128])
    wpt0, wpt1 = sb("wpt0s", [128, 256]), sb("wpt1s", [128, 256])
    bpj0, bpj1 = sb("bpj0s", [128, 1]), sb("bpj1s", [128, 1])
    ysb0, ysb1 = sb("ysb0", [128, NH]), sb("ysb1", [128, NH])

    dma_sem = nc.alloc_semaphore("const_dma")
    nval = 0
    for dst, src in [
        (ks0, ks_d[0]), (ks1, ks_d[1]), (vs, vs_d[:]),
        (qt0, qt_d[0]), (qt1, qt_d[1]), (qpt0, qpt_d[0]), (qpt1, qpt_d[1]),
        (ones8, ones8_d[:]), (w2e, w2e_d[:]), (w2r, w2r_d[:]), (w2g, w2g_d[:]),
        (biasv, biasv_d[:]), (sel1, sel_d[0]), (sel2, sel_d[1]),
        (wpt0, wpt_d[0]), (wpt1, wpt_d[1]), (bpj0, bpj_d[0]), (bpj1, bpj_d[1]),
    ]:
        nc.sync.dma_start(dst[:], src).then_inc(dma_sem, 16)
        nval += 16
    for eng in nc.engines.values():
        eng.wait_ge(dma_sem, nval)

    with tile.TileContext(nc) as tc:
        with (
            tc.tile_pool(name="work", bufs=2) as wp,
            tc.tile_pool(name="psw", bufs=3, space=bass.MemorySpace.PSUM) as psw,
            tc.tile_pool(name="psm", bufs=2, space=bass.MemorySpace.PSUM) as psm,
            tc.tile_pool(name="acc", bufs=1, space=bass.MemorySpace.PSUM) as pacc,
        ):
            psO1 = pacc.tile([128, NH], f32, tag="psO1")
            psO2 = pacc.tile([128, NH], f32, tag="psO2")
            psD = pacc.tile([8, NH], f32, tag="psD")

            for g in range(G):
                kcol = bass.ts(g, 128)
                psA = psw.tile([128, NH], f32, tag="pab")
                psB = psw.tile([128, NH], f32, tag="pab")
                nc.tensor.matmul(psA[:], ks0[:, kcol], qt0[:], start=True, stop=False)
                nc.tensor.matmul(psA[:], ks1[:, kcol], qt1[:], start=False, stop=True)
                nc.tensor.matmul(psB[:], ks0[:, kcol], qpt0[:], start=True, stop=False)
                nc.tensor.matmul(psB[:], ks1[:, kcol], qpt1[:], start=False, stop=True)

                cd = wp.tile([128, NH], f32, tag="cd")
                sd = wp.tile([128, NH], f32, tag="sd")
                sg = wp.tile([128, NH], f32, tag="sg")
                nc.vector.tensor_copy(cd[:], psA[:])
                nc.scalar.activation(sd[:], psA[:], AF.Square)
                nc.scalar.activation(sg[:], psB[:], AF.Square)

                psC = psm.tile([128, NH], f32, tag="pc")
                nc.tensor.matmul(psC[:], w2e[:], cd[:], start=True, stop=False)
                nc.tensor.matmul(psC[:], w2r[:], sd[:], start=False, stop=False)
                nc.tensor.matmul(psC[:], w2g[:], sg[:], start=False, stop=True)

                es = wp.tile([128, NH], f32, tag="es")
                nc.scalar.activation(es[:], psC[:], AF.Exp, bias=biasv[:])

                first, last = g == 0, g == G - 1
                nc.tensor.matmul(psO1[:], vs[:, g * 256:g * 256 + 128], es[:],
                                 start=first, stop=last, skip_group_check=True)
                nc.tensor.matmul(psO2[:], vs[:, g * 256 + 128:g * 256 + 256], es[:],
                                 start=first, stop=last, skip_group_check=True)
                nc.tensor.matmul(psD[:], ones8[:], es[:],
                                 start=first, stop=last, skip_group_check=True)

            rec = wp.tile([8, NH], f32, tag="rec")
            nc.vector.reciprocal(rec[:], psD[:])
            psb1 = psw.tile([128, NH], f32, tag="pab")
            psb2 = psw.tile([128, NH], f32, tag="pab")
            nc.tensor.matmul(psb1[:], sel1[:], rec[:], start=True, stop=True)
            nc.tensor.matmul(psb2[:], sel2[:], rec[:], start=True, stop=True)
            bd1 = wp.tile([128, NH], f32, tag="cd")
            bd2 = wp.tile([128, NH], f32, tag="sd")
            nc.scalar.copy(bd1[:], psb1[:])
            nc.scalar.copy(bd2[:], psb2[:])
            ot1 = wp.tile([128, NH], f32, tag="sg")
            ot2 = wp.tile([128, NH], f32, tag="es")
            nc.vector.tensor_mul(ot1[:], psO1[:], bd1[:])
            nc.vector.tensor_mul(ot2[:], psO2[:], bd2[:])

            for mt in range(2):
                psY = psm.tile([128, NH], f32, tag="pc")
                mcol = bass.ts(mt, 128)
                nc.tensor.matmul(psY[:], wpt0[:, mcol], ot1[:], start=True, stop=False)
                nc.tensor.matmul(psY[:], wpt1[:, mcol], ot2[:], start=False, stop=True)
                nc.scalar.activation((ysb0 if mt == 0 else ysb1)[:], psY[:],
                                     AF.Identity,
                                     bias=(bpj0[:] if mt == 0 else bpj1[:]))

    nc.all_engine_barrier()
    nc.sync.dma_start(yt_d[0], ysb0[:]).then_inc(dma_sem, 16)
    nc.sync.dma_start(yt_d[1], ysb1[:]).then_inc(dma_sem, 16)
    nval += 32
    nc.sync.wait_ge(dma_sem, nval)
    nc.compile()
    return nc


def _host_prep(inputs):
    x = np.asarray(inputs["x"], np.float32)
    w_qkv = np.asarray(inputs["w_qkv"], np.float32)
    b_qkv = np.asarray(inputs["b_qkv"], np.float32)
    qkv = (x.reshape(B * N, C) @ w_qkv.T + b_qkv).reshape(B, N, 3, H, HD)
    qkv = np.ascontiguousarray(qkv.transpose(2, 0, 3, 1, 4))
    q, k, v = qkv[0], qkv[1], qkv[2]          # [B,H,N,HD] f32

    _, Rq = np.linalg.qr(q)
    _, Rk = np.linalg.qr(k)
    eye = np.broadcast_to(np.eye(HD, dtype=np.float32), Rq.shape)
    Rqi = np.linalg.solve(Rq, eye)
    Rki = np.linalg.solve(Rk, eye)
    M = (Rqi @ Rki.transpose(0, 1, 3, 2)).astype(np.float32)
    qp = np.einsum("bhnd,bhde->bhne", q, M).astype(np.float32)

    inv = np.asarray(inputs["bn_gamma"], np.float32) / np.sqrt(
        np.asarray(inputs["bn_var"], np.float32) + BN_EPS)
    cw = np.asarray(inputs["conv_w"], np.float32)
    W2 = cw * inv[None, :]
    bias2 = (np.asarray(inputs["conv_b"], np.float32)
             + (cw * (np.asarray(inputs["bn_beta"], np.float32)
                      - np.asarray(inputs["bn_mean"], np.float32) * inv)[None, :]).sum(1))
    W2e = W2[:, :8] * np.float32(inputs["scale"])
    W2r = W2[:, 8:16] * np.float32(inputs["riem_scale"])
    W2g = W2[:, 16:24] * np.float32(inputs["grassman_scale"])

    w2e_bd = np.kron(np.eye(16, dtype=np.float32), W2e.T).astype(np.float32)
    w2r_bd = np.kron(np.eye(16, dtype=np.float32), W2r.T).astype(np.float32)
    w2g_bd = np.kron(np.eye(16, dtype=np.float32), W2g.T).astype(np.float32)
    biasv = np.tile(bias2, 16).astype(np.float32)[:, None]

    ones8 = np.zeros((128, 8), np.float32)
    for h in range(H):
        ones8[np.arange(16) * 8 + h, h] = 1.0
    sel = np.zeros((2, 8, 128), np.float32)
    for o in range(4):
        sel[0, o, o * 32:(o + 1) * 32] = 1.0
        sel[1, 4 + o, o * 32:(o + 1) * 32] = 1.0

    w_proj = np.asarray(inputs["w_proj"], np.float32)
    wpt = np.ascontiguousarray(w_proj.T.reshape(2, 128, 256))
    bpj = np.asarray(inputs["b_proj"], np.float32).reshape(2, 128, 1)

    per_batch = []
    for b in range(B):
        ks = np.zeros((2, 128, G * 128), np.float32)
        for h in range(H):
            buf = np.zeros((32, G, 128), np.float32)
            buf[:, :, np.arange(16) * 8 + h] = k[b, h].reshape(G, 16, HD).transpose(2, 0, 1)
            ks[h // 4, (h % 4) * 32:(h % 4) * 32 + 32, :] = buf.reshape(32, G * 128)
        vsb = np.zeros((128, G, 256), np.float32)
        for h in range(H):
            vsb[np.arange(16) * 8 + h, :, h * 32:(h + 1) * 32] = \
                v[b, h].reshape(G, 16, HD).transpose(1, 0, 2)
        per_batch.append((ks, vsb.reshape(128, G * 256)))

    in_maps = []
    for core in range(8):
        b, half = core // 2, core % 2
        n0 = half * NH
        qt = np.zeros((2, 128, NH), np.float32)
        qpt = np.zeros((2, 128, NH), np.float32)
        for h in range(H):
            r = (h % 4) * 32
            qt[h // 4, r:r + 32, :] = q[b, h, n0:n0 + NH, :].T
            qpt[h // 4, r:r + 32, :] = qp[b, h, n0:n0 + NH, :].T
        ks, vsb = per_batch[b]
        in_maps.append({
            "qt": qt, "qpt": qpt, "ks": ks, "vs_in": vsb,
            "ones8": ones8, "w2e": w2e_bd, "w2r": w2r_bd, "w2g": w2g_bd,
            "biasv": biasv, "sel": sel, "wpt": wpt, "bpj": bpj,
        })
    return in_maps


def _run(in_maps, trace=False):
    from concourse.bass_utils import run_bass_kernel_spmd
    if "nc" not in _CACHE:
        _CACHE["nc"] = _build_program()
    return run_bass_kernel_spmd(_CACHE["nc"], in_maps, list(range(8)), trace=trace)


def kernel(**inputs):
    in_maps = _host_prep(inputs)
    res = _run(in_maps)
    out = np.empty((B, N, C), np.float32)
    for core in range(8):
        b, half = core // 2, core % 2
        yt = res.results[core]["yt"].reshape(C, NH)
        out[b, half * NH:(half + 1) * NH, :] = yt.T
    return out

